# revision 4
# baseline (speedup 1.0000x reference)
"""Trainium2 kernel for cellpose-style flow integration (grid_sample scan).

Strategy:
  - Host builds a padded "patch table" T[r*2050+c] = the 8 values
    [a00,a01,a10,a11,b00,b01,b10,b11] of the 2x2 bilinear corner patch at
    padded pixel (r,c), pre-scaled by 1024 so the state update happens in
    padded-pixel coordinates; zero pad rows/cols encode zeros-padding.
  - Points are sharded across 8 NeuronCores (32768 each, laid out [128,256]).
  - State u = pt*1024 + 1024.5 (padded pixel coords, clip [0.5, 2048.5]).
    Per iteration and chunk: xi = i32(u - 0.5) (round-to-nearest keeps
    frac in [0,1] -- bilinear continuity makes any tie behavior exact),
    fx = u - xi, q = yi*2050 + xi, per-partition indirect-DMA patch
    gathers, separable lerp, u += s, clip.  4 chunks, with each chunk's
    index computation emitted before the next chunk's gathers so the Pool
    engine (SWDGE emission, the bottleneck) never stalls.
"""
import numpy as np

H = W = 2048
NPTS = 262144
N_CORES = 8
PTS_PER_CORE = NPTS // N_CORES          # 32768
P = 128
F = PTS_PER_CORE // P                   # 256 free elems per partition
PAD = 2050                              # padded table row length
NCHUNK = 4

_compiled = {}


def _build_nc(niter: int):
    import concourse.bass as bass
    import concourse.mybir as mybir
    import concourse.tile as tile
    from concourse import bacc

    f32 = mybir.dt.float32
    bf16 = mybir.dt.bfloat16
    i32 = mybir.dt.int32
    Alu = mybir.AluOpType

    nc = bacc.Bacc("TRN2", target_bir_lowering=False, debug=False,
                   num_devices=N_CORES)
    tab = nc.dram_tensor("tab", [PAD * PAD, 8], f32, kind="ExternalInput").ap()
    p0x = nc.dram_tensor("p0x", [P, F], f32, kind="ExternalInput").ap()
    p0y = nc.dram_tensor("p0y", [P, F], f32, kind="ExternalInput").ap()
    outx = nc.dram_tensor("outx", [P, F], f32, kind="ExternalOutput").ap()
    outy = nc.dram_tensor("outy", [P, F], f32, kind="ExternalOutput").ap()

    FC = F // NCHUNK

    with tile.TileContext(nc) as tc:
        with (
            tc.tile_pool(name="state", bufs=1) as state,
            tc.tile_pool(name="scratch", bufs=NCHUNK + 1) as scratch,
            tc.tile_pool(name="gbuf", bufs=NCHUNK + 1) as gbuf,
        ):
            ux = state.tile([P, F], f32, tag="ux")
            uy = state.tile([P, F], f32, tag="uy")
            nc.gpsimd.dma_start(out=ux[:], in_=p0x[:])
            nc.gpsimd.dma_start(out=uy[:], in_=p0y[:])

            def emit_qi(c):
                """index + frac computation for chunk c; returns tiles."""
                cs = slice(c * FC, (c + 1) * FC)
                uxc, uyc = ux[:, cs], uy[:, cs]
                xi = scratch.tile([P, FC], i32, tag=f"xi{c % NCHUNK}")
                yi = scratch.tile([P, FC], i32, tag=f"yi{c % NCHUNK}")
                fx = scratch.tile([P, FC], f32, tag=f"fx{c % NCHUNK}")
                fy = scratch.tile([P, FC], f32, tag=f"fy{c % NCHUNK}")
                qi = scratch.tile([P, FC], i32, tag=f"qi{c % NCHUNK}")
                nc.vector.tensor_scalar(out=xi[:], in0=uxc, scalar1=0.5,
                                        scalar2=None, op0=Alu.subtract)
                nc.vector.tensor_scalar(out=yi[:], in0=uyc, scalar1=0.5,
                                        scalar2=None, op0=Alu.subtract)
                nc.vector.tensor_tensor(out=fx[:], in0=uxc, in1=xi[:],
                                        op=Alu.subtract)
                nc.vector.tensor_tensor(out=fy[:], in0=uyc, in1=yi[:],
                                        op=Alu.subtract)
                nc.vector.scalar_tensor_tensor(out=qi[:], in0=yi[:],
                                               scalar=2050.0, in1=xi[:],
                                               op0=Alu.mult, op1=Alu.add)
                return qi, fx, fy

            def emit_gather(c, qi):
                g = gbuf.tile([P, FC, 8], f32, tag=f"g{c % NCHUNK}")
                for j in range(FC):
                    nc.gpsimd.indirect_dma_start(
                        out=g[:, j, :],
                        out_offset=None,
                        in_=tab[:, :],
                        in_offset=bass.IndirectOffsetOnAxis(
                            ap=qi[:, j:j + 1], axis=0),
                    )
                return g

            def emit_lerp(c, g, fx, fy):
                cs = slice(c * FC, (c + 1) * FC)
                uxc, uyc = ux[:, cs], uy[:, cs]
                d = scratch.tile([P, FC, 4], f32, tag=f"d{c % NCHUNK}")
                h = scratch.tile([P, FC, 4], f32, tag=f"h{c % NCHUNK}")
                d2 = scratch.tile([P, FC, 2], f32, tag=f"d2{c % NCHUNK}")
                s = scratch.tile([P, FC, 2], f32, tag=f"s{c % NCHUNK}")
                nc.vector.tensor_tensor(out=d[:], in0=g[:, :, 1::2],
                                        in1=g[:, :, 0::2], op=Alu.subtract)
                nc.vector.tensor_tensor(out=d[:], in0=d[:],
                                        in1=fx[:].to_broadcast([P, FC, 4]),
                                        op=Alu.mult)
                nc.vector.tensor_tensor(out=h[:], in0=g[:, :, 0::2],
                                        in1=d[:], op=Alu.add)
                nc.vector.tensor_tensor(out=d2[:], in0=h[:, :, 1::2],
                                        in1=h[:, :, 0::2], op=Alu.subtract)
                nc.vector.tensor_tensor(out=d2[:], in0=d2[:],
                                        in1=fy[:].to_broadcast([P, FC, 2]),
                                        op=Alu.mult)
                nc.vector.tensor_tensor(out=s[:], in0=h[:, :, 0::2],
                                        in1=d2[:], op=Alu.add)
                nc.vector.tensor_tensor(out=uxc, in0=uxc, in1=s[:, :, 0],
                                        op=Alu.add)
                nc.vector.tensor_tensor(out=uyc, in0=uyc, in1=s[:, :, 1],
                                        op=Alu.add)
                nc.vector.tensor_scalar(out=uxc, in0=uxc, scalar1=0.5,
                                        scalar2=2048.5, op0=Alu.max,
                                        op1=Alu.min)
                nc.vector.tensor_scalar(out=uyc, in0=uyc, scalar1=0.5,
                                        scalar2=2048.5, op0=Alu.max,
                                        op1=Alu.min)

            # software-pipelined emission: qi_c / G_c go out before lerp_{c-1}
            # so the Pool gather stream never waits on DVE.
            pend = None  # (c, g, fx, fy) awaiting lerp
            for it in range(niter):
                for c in range(NCHUNK):
                    qi, fx, fy = emit_qi(c)
                    g = emit_gather(c, qi)
                    if pend is not None:
                        emit_lerp(*pend)
                    pend = (c, g, fx, fy)
            emit_lerp(*pend)

            # final: pix = (u - 0.5) * (1023.5/1024)
            ox = state.tile([P, F], f32, tag="ox")
            oy = state.tile([P, F], f32, tag="oy")
            nc.vector.tensor_scalar(out=ox[:], in0=ux[:], scalar1=0.5,
                                    scalar2=0.99951171875, op0=Alu.subtract,
                                    op1=Alu.mult)
            nc.vector.tensor_scalar(out=oy[:], in0=uy[:], scalar1=0.5,
                                    scalar2=0.99951171875, op0=Alu.subtract,
                                    op1=Alu.mult)
            nc.gpsimd.dma_start(out=outx[:], in_=ox[:])
            nc.gpsimd.dma_start(out=outy[:], in_=oy[:])

    nc.compile()
    return nc


def _build_table(dP: np.ndarray) -> np.ndarray:
    """T[r*2050+c, 0:8] = 2x2 patch of (im0,im1)*1024 at padded (r,c)."""
    scale = np.float32(2048.0 / 2047.0)
    im0 = (dP[1] * scale).astype(np.float32)   # adds to pt x (in pixel units)
    im1 = (dP[0] * scale).astype(np.float32)   # adds to pt y
    imp = np.zeros((PAD + 1, PAD + 1, 2), np.float32)
    imp[1:H + 1, 1:W + 1, 0] = im0
    imp[1:H + 1, 1:W + 1, 1] = im1
    T = np.empty((PAD, PAD, 8), np.float32)
    T[:, :, 0] = imp[:PAD, :PAD, 0]       # a00
    T[:, :, 1] = imp[:PAD, 1:, 0]         # a01
    T[:, :, 2] = imp[1:, :PAD, 0]         # a10
    T[:, :, 3] = imp[1:, 1:, 0]           # a11
    T[:, :, 4] = imp[:PAD, :PAD, 1]       # b00
    T[:, :, 5] = imp[:PAD, 1:, 1]         # b01
    T[:, :, 6] = imp[1:, :PAD, 1]         # b10
    T[:, :, 7] = imp[1:, 1:, 1]           # b11
    return T.reshape(PAD * PAD, 8)


def _initial_pts(inds: np.ndarray):
    # u0 = pt0*1024 + 1024.5 with pt0 = inds/2047*2 - 1  ->  inds*(2048/2047) + 0.5
    u0x = (inds[1].astype(np.float64) * (2048.0 / 2047.0) + 0.5).astype(np.float32)
    u0y = (inds[0].astype(np.float64) * (2048.0 / 2047.0) + 0.5).astype(np.float32)
    return u0x, u0y


def kernel(dP: np.ndarray, inds: np.ndarray, niter) -> np.ndarray:
    from concourse.bass_utils import run_bass_kernel_spmd

    niter = int(niter)
    dP = np.asarray(dP, np.float32)
    inds = np.asarray(inds)

    if niter not in _compiled:
        _compiled[niter] = _build_nc(niter)
    nc = _compiled[niter]

    T = _build_table(dP)
    ptx, pty = _initial_pts(inds)

    in_maps = []
    for i in range(N_CORES):
        sl = slice(i * PTS_PER_CORE, (i + 1) * PTS_PER_CORE)
        in_maps.append({
            "tab": T,
            "p0x": ptx[sl].reshape(P, F),
            "p0y": pty[sl].reshape(P, F),
        })

    res = run_bass_kernel_spmd(nc, in_maps, list(range(N_CORES)))

    out = np.empty((2, NPTS), np.float32)
    for i in range(N_CORES):
        sl = slice(i * PTS_PER_CORE, (i + 1) * PTS_PER_CORE)
        out[0, sl] = res.results[i]["outy"].reshape(-1)
        out[1, sl] = res.results[i]["outx"].reshape(-1)
    return out
